# revision 4
# baseline (speedup 1.0000x reference)
"""MoE FFN (8 experts, top-2, raw-logit combine) — Trainium2 Bass kernel,
expert-parallel across 8 NeuronCores.

One expert per core. The host performs all routing ("all-to-all dispatch"):
gate + top-2 in exact fp32, token gather per expert, and the final
scatter-add combine. Each core runs a pure dense 2-layer MLP over C capacity
slots in fp16, with every operand pre-tiled on the host so the device needs
zero transposes:

  h^T[m] = gelu(sum_k W1[k,m]^T @ x^T[k] + b1[m])   24 m-tiles, PSUM acc over k
  y^T[d] = sum_m W2[m,d]^T @ h^T[m]                  6 d-tiles, PSUM acc over m

The device returns raw y^T; the host adds b2, scales rows by the raw top-2
gate scores, and scatter-adds the 8 per-expert partials into the output.
Tokens routed past an expert's C capacity slots (rare; capacity covers the
observed max load) are computed exactly on the host in float64.

Layout/perf notes (measured on HW):
- All input DMAs ride the SP HWDGE ring in consumption order (x halves,
  W1 groups, then W2 groups); stores ride the ACT ring so the SP FIFO never
  stalls the next iteration's loads behind an end-of-iteration store.
- W1 streams in 4-m-tile batches (first batch split per-m so PE starts
  ~3.5us in); W2 is consumed group-major in two d-halves so it is used in
  DMA-arrival order and half the outputs drain early.
- The bench loop unrolls x8 with a PE branch-prefetch hint: the back-edge
  barrier, head, and tail amortize, and tile pools rotate across copies.
"""

import os
from contextlib import ExitStack

import numpy as np

import concourse.bacc as bacc
import concourse.bass as bass
import concourse.mybir as mybir
import concourse.tile as tile
from concourse.bass_utils import run_bass_kernel_spmd

P = 128
T, D, H, E = 1024, 768, 3072, 8
KD, MH = D // P, H // P  # 6, 24
C = 288  # capacity slots per expert; multiple of 16 (32B fp16 line alignment)
G1 = 4  # W1 m-tiles per DMA batch
G2 = 6  # W2 m-tiles per DMA batch
UNROLL = 8
F32 = mybir.dt.float32
F16 = mybir.dt.float16
PSUM = bass.MemorySpace.PSUM

LAST_RESULTS = None
VARIANT = "v2"


def _build_v2(reps=1):
    act_func = mybir.ActivationFunctionType.Gelu
    nc = bacc.Bacc("TRN2", target_bir_lowering=False, debug=False)

    xct_d = nc.dram_tensor("xct", [P, KD, C], F16, kind="ExternalInput").ap()
    w1_d = nc.dram_tensor(
        "w1t", [MH // G1, P, G1, D], F16, kind="ExternalInput"
    ).ap()
    w2_d = nc.dram_tensor(
        "w2t", [MH // G2, P, G2, D], F16, kind="ExternalInput"
    ).ap()
    b1_d = nc.dram_tensor("b1t", [P, MH], F32, kind="ExternalInput").ap()
    out_d = nc.dram_tensor("yt", [KD, P, C], F32, kind="ExternalOutput").ap()

    with tile.TileContext(nc) as tc, ExitStack() as ctx:
        consts = ctx.enter_context(tc.tile_pool(name="consts", bufs=2))
        w1p = ctx.enter_context(tc.tile_pool(name="w1p", bufs=4))
        w2p = ctx.enter_context(tc.tile_pool(name="w2p", bufs=2))
        xp = ctx.enter_context(tc.tile_pool(name="xp", bufs=2))
        hp = ctx.enter_context(tc.tile_pool(name="hp", bufs=2))
        yp = ctx.enter_context(tc.tile_pool(name="yp", bufs=3))
        # all 8 PSUM banks: 5 rotating h accumulators (PE never waits on the
        # ACT gelu evacuation of an earlier h tile) + 3 y accumulators shared
        # across the two W2 d-halves
        psh = ctx.enter_context(tc.tile_pool(name="psh", bufs=5, space=PSUM))
        psy = ctx.enter_context(tc.tile_pool(name="psy", bufs=1, space=PSUM))

        def _body():
            xsb = xp.tile([P, KD, C], F16, tag="x", name="xsb")
            nc.sync.dma_start(xsb[:, 0:3, :], xct_d[:, 0:3, :])
            nc.sync.dma_start(xsb[:, 3:KD, :], xct_d[:, 3:KD, :])
            b1s = consts.tile([P, MH], F32, tag="b1", name="b1s")
            nc.sync.dma_start(b1s[:], b1_d[:])

            # W2 group tiles, all resident (consumed group-major below)
            w2s = [
                w2p.tile([P, G2, D], F16, tag=f"w2_{g}", name=f"w2s{g}")
                for g in range(MH // G2)
            ]

            # W1 stage: h^T[m] = gelu(W1^T x^T + b1), streamed over m
            hts = []
            w1gs = {}
            for m in range(MH):
                g, j = divmod(m, G1)
                if j == 0:
                    w1gs[g] = w1p.tile([P, G1, D], F16, tag="w1", name=f"w1g{g}")
                    if g == 0:  # finer grain so m=0 weights land first
                        for jj in range(G1):
                            nc.sync.dma_start(
                                w1gs[g][:, jj, :], w1_d[g][:, jj, :]
                            )
                    else:
                        nc.sync.dma_start(w1gs[g][:], w1_d[g])
                w1s = w1gs[g]
                hps = psh.tile([P, C], F32, tag="h", name=f"hps{m}")
                for k in range(KD):
                    nc.tensor.matmul(
                        hps[:],
                        w1s[:, j, k * P : (k + 1) * P],
                        xsb[:, k, :],
                        start=(k == 0),
                        stop=(k == KD - 1),
                    )
                ht = hp.tile([P, C], F16, tag=f"h{m}", name=f"ht{m}")
                nc.scalar.activation(
                    ht[:], hps[:], act_func, bias=b1s[:, m : m + 1], scale=1.0
                )
                hts.append(ht)

            # W2 loads issued after the whole W1 stream on the same (SP)
            # ring: DMA service order = issue order, so W1 tiles (which gate
            # PE first) always win.
            for g in range(MH // G2):
                nc.sync.dma_start(w2s[g][:], w2_d[g])

            # W2 stage: y^T[d] = sum_m W2[m,d]^T h^T[m].
            # Group-major accumulation in two d-halves: consumes W2 group g
            # in DMA-arrival order; the first half's outputs drain while the
            # second half computes.
            ND2 = KD // 2
            for half in range(2):
                yps_l = [
                    psy.tile([P, C], F32, tag=f"y_{i}", name=f"yps{half}_{i}")
                    for i in range(ND2)
                ]
                for g in range(MH // G2):
                    for i in range(ND2):
                        d = half * ND2 + i
                        for j in range(G2):
                            m = g * G2 + j
                            nc.tensor.matmul(
                                yps_l[i][:],
                                w2s[g][:, j, d * P : (d + 1) * P],
                                hts[m][:],
                                start=(g == 0 and j == 0),
                                stop=(g == MH // G2 - 1 and j == G2 - 1),
                            )
                for i in range(ND2):
                    d = half * ND2 + i
                    ysb = yp.tile([P, C], F32, tag="y", name=f"ysb{d}")
                    nc.vector.tensor_copy(ysb[:], yps_l[i][:])
                    # stores ride the ACT HWDGE ring so the SP ring's FIFO
                    # never makes the next iteration's loads wait on them
                    nc.scalar.dma_start(out_d[d], ysb[:])

        if reps > 1:
            tc.For_i_unrolled_general(
                0,
                reps,
                1,
                lambda iv, n: [_body() for _ in range(n)],
                max_unroll=UNROLL,
                hint_engines=(mybir.EngineType.PE,),
            )
        else:
            _body()

    nc.compile()
    return nc


def _route(x, Wg, bg):
    x2 = np.ascontiguousarray(np.asarray(x, np.float32).reshape(T, D))
    gate = x2 @ np.asarray(Wg, np.float32) + np.asarray(bg, np.float32)
    top2 = np.argsort(-gate, axis=1)[:, :2]
    return x2, gate, top2


def make_v2_in_maps(x, Wg, bg, W1, b1, W2, b2):
    x2, gate, top2 = _route(x, Wg, bg)
    in_maps = []
    meta = []
    for e in range(E):
        sel = (top2 == e).any(axis=1)
        idx = np.nonzero(sel)[0]
        idx, idx_over = idx[:C], idx[C:]  # overflow handled on host (rare)
        xc = np.zeros((C, D), np.float16)
        xc[: len(idx)] = x2[idx]
        # [C, D] -> [P, KD, C]: xct[p, k, c] = xc[c, k*P+p]
        xct = np.ascontiguousarray(xc.T.reshape(KD, P, C).transpose(1, 0, 2))
        w1 = np.asarray(W1[e], np.float16)  # [D, H]
        # lhsT tiles grouped for batched DMA: tile (m=g*G1+j, k) is
        # W1[kP:(k+1)P, mP:(m+1)P], laid out [g, p, j, (k q)]
        w1t = (
            w1.reshape(KD, P, MH, P)
            .transpose(2, 1, 0, 3)
            .reshape(MH // G1, G1, P, D)
            .transpose(0, 2, 1, 3)
        )
        w1t = np.ascontiguousarray(w1t)
        w2t = (
            np.asarray(W2[e], np.float16)
            .reshape(MH // G2, G2, P, D)
            .transpose(0, 2, 1, 3)
        )
        w2t = np.ascontiguousarray(w2t)
        b1t = np.ascontiguousarray(np.asarray(b1[e], np.float32).reshape(MH, P).T)
        in_maps.append(dict(xct=xct, w1t=w1t, w2t=w2t, b1t=b1t))
        meta.append((idx, gate[idx, e], idx_over, gate[idx_over, e]))
    return in_maps, meta


def _erf(z):
    try:
        from scipy.special import erf

        return erf(z)
    except ImportError:
        import math

        return np.vectorize(math.erf)(z)


def finish_v2(results, meta, x, W1, b1, W2, b2):
    out = np.zeros((T, D), np.float64)
    b2 = np.asarray(b2, np.float64)
    x2 = np.asarray(x, np.float64).reshape(T, D)
    for e in range(E):
        idx, scores, idx_over, scores_over = meta[e]
        yt = np.asarray(results[e]["yt"], np.float64)  # [KD, P, C]
        y = yt.reshape(D, C).T  # [C, D]
        out[idx] += (y[: len(idx)] + b2[e]) * scores[:, None]
        if len(idx_over):  # exact host fallback for capacity overflow
            ho = x2[idx_over] @ np.asarray(W1[e], np.float64) + np.asarray(
                b1[e], np.float64
            )
            ho = ho * 0.5 * (1.0 + _erf(ho / np.sqrt(2.0)))
            yo = ho @ np.asarray(W2[e], np.float64) + b2[e]
            out[idx_over] += yo * scores_over[:, None]
    return out.astype(np.float32).reshape(1, T, D)


_BUILT = {}


def kernel(x, Wg, bg, W1, b1, W2, b2):
    global LAST_RESULTS
    if "v2" not in _BUILT:
        _BUILT["v2"] = _build_v2()
    nc = _BUILT["v2"]
    in_maps, meta = make_v2_in_maps(x, Wg, bg, W1, b1, W2, b2)
    rr = run_bass_kernel_spmd(nc, in_maps, core_ids=list(range(E)))
    LAST_RESULTS = rr
    return finish_v2(rr.results, meta, x, W1, b1, W2, b2)


# revision 5
# speedup vs baseline: 1.2074x; 1.2074x over previous
"""MoE FFN (8 experts, top-2, raw-logit combine) — Trainium2 Bass kernel,
expert-parallel across 8 NeuronCores.

One expert per core. The host performs all routing ("all-to-all dispatch"):
gate + top-2 in exact fp32, token gather per expert, and the final
scatter-add combine. Each core runs a pure dense 2-layer MLP over C capacity
slots in fp16, with every operand pre-tiled on the host so the device needs
zero transposes:

  h^T[m] = gelu(sum_k W1[k,m]^T @ x^T[k] + b1[m])   24 m-tiles, PSUM acc over k
  y^T[d] = sum_m W2[m,d]^T @ h^T[m]                  6 d-tiles, PSUM acc over m

The device returns raw y^T; the host adds b2, scales rows by the raw top-2
gate scores, and scatter-adds the 8 per-expert partials into the output.
Tokens routed past an expert's C capacity slots (rare; capacity covers the
observed max load) are computed exactly on the host in float64.

Layout/perf notes (measured on HW):
- All input DMAs ride the SP HWDGE ring in consumption order (x halves,
  W1 groups, then W2 groups); stores ride the ACT ring so the SP FIFO never
  stalls the next iteration's loads behind an end-of-iteration store.
- W1 streams in 4-m-tile batches (first batch split per-m so PE starts
  ~3.5us in); W2 is consumed group-major in two d-halves so it is used in
  DMA-arrival order and half the outputs drain early.
- The bench loop unrolls x8 with a PE branch-prefetch hint: the back-edge
  barrier, head, and tail amortize, and tile pools rotate across copies.
"""

import os
from contextlib import ExitStack

import numpy as np

import concourse.bacc as bacc
import concourse.bass as bass
import concourse.mybir as mybir
import concourse.tile as tile
from concourse.bass_utils import run_bass_kernel_spmd

P = 128
T, D, H, E = 1024, 768, 3072, 8
KD, MH = D // P, H // P  # 6, 24
C = 288  # capacity slots per expert; multiple of 16 (32B fp16 line alignment)
G1 = 4  # W1 m-tiles per DMA batch
G2 = 12  # W2 m-tiles per DMA batch
UNROLL = 8
F32 = mybir.dt.float32
F16 = mybir.dt.float16
PSUM = bass.MemorySpace.PSUM

LAST_RESULTS = None
VARIANT = "v2"


def _build_v2(reps=1):
    act_func = mybir.ActivationFunctionType.Gelu
    nc = bacc.Bacc("TRN2", target_bir_lowering=False, debug=False)

    xct_d = nc.dram_tensor("xct", [P, KD, C], F16, kind="ExternalInput").ap()
    w1_d = nc.dram_tensor(
        "w1t", [MH // G1, P, G1, D], F16, kind="ExternalInput"
    ).ap()
    w2_d = nc.dram_tensor(
        "w2t", [MH // G2, P, G2, D], F16, kind="ExternalInput"
    ).ap()
    b1_d = nc.dram_tensor("b1t", [P, MH], F32, kind="ExternalInput").ap()
    out_d = nc.dram_tensor("yt", [KD, P, C], F32, kind="ExternalOutput").ap()

    with tile.TileContext(nc) as tc, ExitStack() as ctx:
        consts = ctx.enter_context(tc.tile_pool(name="consts", bufs=2))
        w1p = ctx.enter_context(tc.tile_pool(name="w1p", bufs=6))
        w2p = ctx.enter_context(tc.tile_pool(name="w2p", bufs=2))
        xp = ctx.enter_context(tc.tile_pool(name="xp", bufs=2))
        hp = ctx.enter_context(tc.tile_pool(name="hp", bufs=3))
        yp = ctx.enter_context(tc.tile_pool(name="yp", bufs=3))
        # all 8 PSUM banks: 5 rotating h accumulators (PE never waits on the
        # ACT gelu evacuation of an earlier h tile) + 3 y accumulators shared
        # across the two W2 d-halves
        psh = ctx.enter_context(tc.tile_pool(name="psh", bufs=5, space=PSUM))
        psy = ctx.enter_context(tc.tile_pool(name="psy", bufs=1, space=PSUM))

        def _body():
            xsb = xp.tile([P, KD, C], F16, tag="x", name="xsb")
            nc.sync.dma_start(xsb[:, 0:3, :], xct_d[:, 0:3, :])
            nc.sync.dma_start(xsb[:, 3:KD, :], xct_d[:, 3:KD, :])
            b1s = consts.tile([P, MH], F32, tag="b1", name="b1s")
            nc.sync.dma_start(b1s[:], b1_d[:])

            # W2 group tiles, all resident (consumed group-major below)
            w2s = [
                w2p.tile([P, G2, D], F16, tag=f"w2_{g}", name=f"w2s{g}")
                for g in range(MH // G2)
            ]

            # W1 stage: h^T[m] = gelu(W1^T x^T + b1), streamed over m
            hts = []
            w1gs = {}
            for m in range(MH):
                g, j = divmod(m, G1)
                if j == 0:
                    w1gs[g] = w1p.tile([P, G1, D], F16, tag="w1", name=f"w1g{g}")
                    if g == 0:  # finer grain so m=0 weights land first
                        for jj in range(G1):
                            nc.sync.dma_start(
                                w1gs[g][:, jj, :], w1_d[g][:, jj, :]
                            )
                    else:
                        nc.sync.dma_start(w1gs[g][:], w1_d[g])
                w1s = w1gs[g]
                hps = psh.tile([P, C], F32, tag="h", name=f"hps{m}")
                for k in range(KD):
                    nc.tensor.matmul(
                        hps[:],
                        w1s[:, j, k * P : (k + 1) * P],
                        xsb[:, k, :],
                        start=(k == 0),
                        stop=(k == KD - 1),
                    )
                ht = hp.tile([P, C], F16, tag=f"h{m}", name=f"ht{m}")
                nc.scalar.activation(
                    ht[:], hps[:], act_func, bias=b1s[:, m : m + 1], scale=1.0
                )
                hts.append(ht)

            # W2 loads issued after the whole W1 stream on the same (SP)
            # ring: DMA service order = issue order, so W1 tiles (which gate
            # PE first) always win.
            for g in range(MH // G2):
                nc.sync.dma_start(w2s[g][:], w2_d[g])

            # W2 stage: y^T[d] = sum_m W2[m,d]^T h^T[m].
            # Group-major accumulation in two d-halves: consumes W2 group g
            # in DMA-arrival order; the first half's outputs drain while the
            # second half computes.
            ND2 = KD // 2
            for half in range(2):
                yps_l = [
                    psy.tile([P, C], F32, tag=f"y_{i}", name=f"yps{half}_{i}")
                    for i in range(ND2)
                ]
                for g in range(MH // G2):
                    for i in range(ND2):
                        d = half * ND2 + i
                        for j in range(G2):
                            m = g * G2 + j
                            nc.tensor.matmul(
                                yps_l[i][:],
                                w2s[g][:, j, d * P : (d + 1) * P],
                                hts[m][:],
                                start=(g == 0 and j == 0),
                                stop=(g == MH // G2 - 1 and j == G2 - 1),
                            )
                for i in range(ND2):
                    d = half * ND2 + i
                    ysb = yp.tile([P, C], F32, tag="y", name=f"ysb{d}")
                    nc.vector.tensor_copy(ysb[:], yps_l[i][:])
                    # stores ride the ACT HWDGE ring so the SP ring's FIFO
                    # never makes the next iteration's loads wait on them
                    nc.scalar.dma_start(out_d[d], ysb[:])

        if reps > 1:
            tc.For_i_unrolled_general(
                0,
                reps,
                1,
                lambda iv, n: [_body() for _ in range(n)],
                max_unroll=UNROLL,
                hint_engines=(mybir.EngineType.PE,),
            )
        else:
            _body()

    nc.compile()
    return nc


def _route(x, Wg, bg):
    x2 = np.ascontiguousarray(np.asarray(x, np.float32).reshape(T, D))
    gate = x2 @ np.asarray(Wg, np.float32) + np.asarray(bg, np.float32)
    top2 = np.argsort(-gate, axis=1)[:, :2]
    return x2, gate, top2


def make_v2_in_maps(x, Wg, bg, W1, b1, W2, b2):
    x2, gate, top2 = _route(x, Wg, bg)
    in_maps = []
    meta = []
    for e in range(E):
        sel = (top2 == e).any(axis=1)
        idx = np.nonzero(sel)[0]
        idx, idx_over = idx[:C], idx[C:]  # overflow handled on host (rare)
        xc = np.zeros((C, D), np.float16)
        xc[: len(idx)] = x2[idx]
        # [C, D] -> [P, KD, C]: xct[p, k, c] = xc[c, k*P+p]
        xct = np.ascontiguousarray(xc.T.reshape(KD, P, C).transpose(1, 0, 2))
        w1 = np.asarray(W1[e], np.float16)  # [D, H]
        # lhsT tiles grouped for batched DMA: tile (m=g*G1+j, k) is
        # W1[kP:(k+1)P, mP:(m+1)P], laid out [g, p, j, (k q)]
        w1t = (
            w1.reshape(KD, P, MH, P)
            .transpose(2, 1, 0, 3)
            .reshape(MH // G1, G1, P, D)
            .transpose(0, 2, 1, 3)
        )
        w1t = np.ascontiguousarray(w1t)
        w2t = (
            np.asarray(W2[e], np.float16)
            .reshape(MH // G2, G2, P, D)
            .transpose(0, 2, 1, 3)
        )
        w2t = np.ascontiguousarray(w2t)
        b1t = np.ascontiguousarray(np.asarray(b1[e], np.float32).reshape(MH, P).T)
        in_maps.append(dict(xct=xct, w1t=w1t, w2t=w2t, b1t=b1t))
        meta.append((idx, gate[idx, e], idx_over, gate[idx_over, e]))
    return in_maps, meta


def _erf(z):
    try:
        from scipy.special import erf

        return erf(z)
    except ImportError:
        import math

        return np.vectorize(math.erf)(z)


def finish_v2(results, meta, x, W1, b1, W2, b2):
    out = np.zeros((T, D), np.float64)
    b2 = np.asarray(b2, np.float64)
    x2 = np.asarray(x, np.float64).reshape(T, D)
    for e in range(E):
        idx, scores, idx_over, scores_over = meta[e]
        yt = np.asarray(results[e]["yt"], np.float64)  # [KD, P, C]
        y = yt.reshape(D, C).T  # [C, D]
        out[idx] += (y[: len(idx)] + b2[e]) * scores[:, None]
        if len(idx_over):  # exact host fallback for capacity overflow
            ho = x2[idx_over] @ np.asarray(W1[e], np.float64) + np.asarray(
                b1[e], np.float64
            )
            ho = ho * 0.5 * (1.0 + _erf(ho / np.sqrt(2.0)))
            yo = ho @ np.asarray(W2[e], np.float64) + b2[e]
            out[idx_over] += yo * scores_over[:, None]
    return out.astype(np.float32).reshape(1, T, D)


_BUILT = {}


def kernel(x, Wg, bg, W1, b1, W2, b2):
    global LAST_RESULTS
    if "v2" not in _BUILT:
        _BUILT["v2"] = _build_v2()
    nc = _BUILT["v2"]
    in_maps, meta = make_v2_in_maps(x, Wg, bg, W1, b1, W2, b2)
    rr = run_bass_kernel_spmd(nc, in_maps, core_ids=list(range(E)))
    LAST_RESULTS = rr
    return finish_v2(rr.results, meta, x, W1, b1, W2, b2)
